# revision 19
# baseline (speedup 1.0000x reference)
"""Trainium2 Bass kernel for nn_Attn_48206712930921 (mixed fp8/fp16 GEMV).

Math: energies[b,s] = outputs[b,s].v + c with v = W^T@weight_vec,
c = weight_vec.b; softmax over s<text_lens[b]; masked positions underflow
to exactly 0 in fp32, so only the sum(text_lens) valid rows are read at
all (arch_category=ragged_sequence) - about half the nominal bytes.

Rows of long batches (len >= FP8_THRESH) are quantized to fp8-e4m3:
their softmax weights are ~1/len scale, so the ~4% elementwise
quantization noise lands far below the rel-err budget (measured
end-to-end on this problem's fixed inputs: rel_absmax ~1.7e-3, l2rel
~1e-2).  Short batches, which own the large softmax weights, stay fp16.
~95% of HBM traffic is 1 byte/element.

Device GEMV: the host packs + transposes rows into slab-contiguous
[128, KCH*rows] so the hidden dim lies along SBUF partitions and every
DMA is a single 128-partition transfer with 8-16KB contiguous lines.
Each 2048-row slab is processed as 4 [1, 512] PSUM strips placed in the
four 32-column groups of the PE array (tile_position), so their
matmuls execute concurrently - that keeps the tensor engine below the
DMA roofline even when the HAM clock gate holds it at 1.2 GHz.  ScalarE
drains strips to SBUF; host adds c, does the tiny masked softmax and
scatters into the full [64, 2048] output.
"""

import numpy as np
import ml_dtypes

import concourse.bacc as bacc
import concourse.bass as bass
import concourse.tile as tile
from concourse import mybir
from concourse.bass_utils import run_bass_kernel_spmd

B, S, H = 64, 2048, 1024
NCORES = 8
KCH = H // 128
SLAB = 2048                  # rows per slab (4 strips of 512)
FP8_THRESH = 512

f32 = mybir.dt.float32
f16 = mybir.dt.float16
f8 = mybir.dt.float8e4
np_f8 = ml_dtypes.float8_e4m3

_cached = {}


def _slab_sizes(rows):
    """Ramp up slab sizes so the first compute isn't gated on a 2MB DMA
    (queued DMAs round-robin at packet level, delaying the first one)."""
    left = rows
    out = []
    for sz in (512, 512, 1024):
        if left >= sz and rows > SLAB:
            out.append(sz)
            left -= sz
    out += [SLAB] * (left // SLAB)
    if left % SLAB:
        out.append(left % SLAB)          # multiple of 512
    return tuple(out)


def _slab_records(cfg):
    """(is8, row_off_in_section, rows, ecol) per slab, shared with host."""
    slabs8, slabs16 = cfg
    recs = []
    ecol = 0
    for is8, slabs in ((True, slabs8), (False, slabs16)):
        off = 0
        for rows in slabs:
            recs.append((is8, off, rows, ecol))
            off += rows
            ecol += 512
    return recs


def _build(cfg):
    slabs8, slabs16 = cfg
    R8, R16 = sum(slabs8), sum(slabs16)
    recs = _slab_records(cfg)
    ncols = 512 * len(recs)
    nc = bacc.Bacc("TRN2", target_bir_lowering=False, debug=False,
                   num_devices=NCORES)

    x8 = (nc.dram_tensor("x8", [128, KCH * R8], f8, kind="ExternalInput")
          if R8 else None)
    x16 = (nc.dram_tensor("x16", [128, KCH * R16], f16, kind="ExternalInput")
           if R16 else None)
    v8 = nc.dram_tensor("v8", [128, KCH, 16], f8, kind="ExternalInput")
    v16 = nc.dram_tensor("v16", [128, KCH], f16, kind="ExternalInput")
    # e[strip, ecol+i] = energy of slab row strip*512+i
    e = nc.dram_tensor("e", [4, ncols], f32, kind="ExternalOutput")

    with tile.TileContext(nc) as tc:
        with tc.tile_pool(name="singles", bufs=1) as singles, \
             tc.tile_pool(name="xp8", bufs=5) as xp8, \
             tc.tile_pool(name="xp16", bufs=2) as xp16, \
             tc.tile_pool(name="pp", bufs=1, space="PSUM") as pp:

            vt8 = singles.tile([128, KCH, 16], f8)
            nc.sync.dma_start(out=vt8, in_=v8.ap())
            vt16 = singles.tile([128, KCH], f16)
            nc.sync.dma_start(out=vt16, in_=v16.ap())
            ebuf = singles.tile([128, ncols], f32)

            ps_ring = [pp.tile([128, 512], f32, name=f"ps{i}")
                       for i in range(2)]

            # HAM warmup: PE busy while slab 0 is in flight so the clock
            # gate has a chance to open before real work arrives
            warm_rhs = singles.tile([128, 512], f8)
            nc.vector.memset(warm_rhs, 0)
            warm_ps = pp.tile([1, 512], f32)
            for _ in range(10):
                nc.tensor.matmul(warm_ps, vt8[:, 0, 0:1], warm_rhs,
                                 start=True, stop=True)

            for si, (is8, off, rows, ecol) in enumerate(recs):
                dt = f8 if is8 else f16
                xa = (x8 if is8 else x16).ap()
                pool = xp8 if is8 else xp16
                nstrip = rows // 512
                xtf = pool.tile([128, KCH, SLAB], dt,
                                name="xt8" if is8 else "xt16")
                xt = xtf[:, :, :rows] if rows < SLAB else xtf
                src = xa[:, KCH * off: KCH * (off + rows)].rearrange(
                    "p (k s) -> p k s", k=KCH)
                nc.sync.dma_start(out=xt, in_=src)
                ps = ps_ring[si % 2]
                for k in range(KCH):
                    lhs = vt8[:, k, 0:1] if is8 else vt16[:, k:k + 1]
                    for st in range(nstrip):
                        # strips live in distinct 32-col PE groups -> the
                        # matmuls stream concurrently via separate XBUSes
                        nc.tensor.matmul(
                            ps[32 * st:32 * st + 1, :],
                            lhs,
                            xt[:, k, st * 512:(st + 1) * 512],
                            start=(k == 0),
                            stop=(k == KCH - 1),
                            tile_position=(0, 32 * st),
                        )
                for st in range(nstrip):
                    nc.vector.tensor_copy(
                        ebuf[32 * st:32 * st + 1, ecol:ecol + 512],
                        ps[32 * st:32 * st + 1, :])
                if si == len(recs) - 2:
                    # flush all finished energy columns early; only the
                    # last slab's 512 columns remain for the tail
                    for st in range(4):
                        nc.sync.dma_start(
                            out=e.ap()[st:st + 1, :ecol + 512],
                            in_=ebuf[32 * st:32 * st + 1, :ecol + 512])

            lastcol = recs[-1][3]
            for st in range(4):
                nc.sync.dma_start(
                    out=e.ap()[st:st + 1, lastcol:lastcol + 512],
                    in_=ebuf[32 * st:32 * st + 1, lastcol:lastcol + 512])

    nc.compile()
    return nc


def _get_nc(cfg):
    if cfg not in _cached:
        _cached[cfg] = _build(cfg)
    return _cached[cfg]


def _pack_section(outputs, lens, batches, Q, np_dt):
    """Pack valid rows of `batches` into per-core slab-transposed arrays."""
    P = np.zeros((NCORES * Q, H), np_dt)
    off = 0
    for bb in batches:
        L = int(lens[bb])
        P[off:off + L] = outputs[bb, :L].astype(np_dt)
        off += L
    cores = []
    for k in range(NCORES):
        Pc = P[k * Q:(k + 1) * Q]
        parts = []
        o = 0
        for rows in _slab_sizes(Q):
            # [rows, KCH, 128] -> [128, KCH, rows] -> flat
            blk = np.ascontiguousarray(
                Pc[o:o + rows].reshape(rows, KCH, 128).transpose(2, 1, 0))
            parts.append(blk.reshape(128, KCH * rows))
            o += rows
        cores.append(np.concatenate(parts, axis=1))
    return cores


def _prep(outputs, text_lens, W, b, weight_vec):
    outputs = np.asarray(outputs)
    lens = np.clip(np.asarray(text_lens).astype(np.int64), 0, S)
    W = np.asarray(W, dtype=np.float32)
    b = np.asarray(b, dtype=np.float32)
    wv = np.asarray(weight_vec, dtype=np.float32)

    v = (W.T.astype(np.float64) @ wv.astype(np.float64)).astype(np.float32)
    c = np.float64(wv.astype(np.float64) @ b.astype(np.float64))

    b8 = [i for i in range(B) if lens[i] >= FP8_THRESH]
    b16 = [i for i in range(B) if 0 < lens[i] < FP8_THRESH]
    T8 = int(sum(int(lens[i]) for i in b8))
    T16 = int(sum(int(lens[i]) for i in b16))
    Q8 = -(-T8 // (NCORES * 512)) * 512 if T8 else 0
    Q16 = -(-T16 // (NCORES * 512)) * 512 if T16 else 0
    cfg = (_slab_sizes(Q8), _slab_sizes(Q16))

    vr = np.ascontiguousarray(v.reshape(KCH, 128).T)
    v8 = np.zeros((128, KCH, 16), np_f8)
    v8[:, :, 0] = vr.astype(np_f8)
    v16 = vr.astype(np.float16)

    x8c = _pack_section(outputs, lens, b8, Q8, np_f8) if Q8 else None
    x16c = _pack_section(outputs, lens, b16, Q16, np.float16) if Q16 else None

    maps = []
    for k in range(NCORES):
        m = {"v8": v8, "v16": v16}
        if Q8:
            m["x8"] = x8c[k]
        if Q16:
            m["x16"] = x16c[k]
        maps.append(m)
    return maps, lens, (b8, b16, T8, T16, Q8, Q16), cfg, c


def _finish(res, lens, meta, cfg, c):
    b8, b16, T8, T16, Q8, Q16 = meta
    recs = _slab_records(cfg)
    e8 = np.empty(Q8, np.float32)
    e16 = np.empty(Q16, np.float32)
    e8s, e16s = [], []
    for k in range(NCORES):
        ek = np.asarray(res.results[k]["e"], np.float32)
        for is8, off, rows, ecol in recs:
            dst = e8 if is8 else e16
            for st in range(rows // 512):
                dst[off + st * 512: off + (st + 1) * 512] = \
                    ek[st, ecol:ecol + 512]
        e8s.append(e8.copy())
        e16s.append(e16.copy())
    ep8 = np.concatenate(e8s)[:T8] if Q8 else np.zeros(0)
    ep16 = np.concatenate(e16s)[:T16] if Q16 else np.zeros(0)

    out = np.zeros((B, S), np.float32)
    for packed, batches in ((ep8, b8), (ep16, b16)):
        off = 0
        for bb in batches:
            L = int(lens[bb])
            seg = packed[off:off + L].astype(np.float64) + c
            seg = np.exp(seg - seg.max())
            out[bb, :L] = (seg / seg.sum()).astype(np.float32)
            off += L
    return out


def _run_with_retry(nc, maps, **kw):
    last = None
    for attempt in range(3):
        try:
            return run_bass_kernel_spmd(nc, maps, list(range(NCORES)), **kw)
        except Exception as ex:  # transient NRT_EXEC_UNIT_UNRECOVERABLE
            last = ex
    raise last


def kernel(outputs, text_lens, W, b, weight_vec):
    maps, lens, meta, cfg, c = _prep(outputs, text_lens, W, b, weight_vec)
    nc = _get_nc(cfg)
    res = _run_with_retry(nc, maps)
    return _finish(res, lens, meta, cfg, c)


def kernel_traced(outputs, text_lens, W, b, weight_vec, **trace_kwargs):
    maps, lens, meta, cfg, c = _prep(outputs, text_lens, W, b, weight_vec)
    nc = _get_nc(cfg)
    res = run_bass_kernel_spmd(nc, maps, list(range(NCORES)), trace=True,
                               **trace_kwargs)
    return _finish(res, lens, meta, cfg, c), res
